# revision 36
# baseline (speedup 1.0000x reference)
"""Trainium2 Bass kernel for a 2-layer GAT (PyG GATConv, concat=False, 3 heads).

Strategy (8 NeuronCores, SPMD):
  * Nodes are range-sharded across the 8 cores (6250 nodes each). Edges are
    partitioned by destination into aligned 128-node dst blocks; self-loops
    appended on the host; per-block edge lists padded to CPB chunks of 128.
  * Per layer, each core computes an augmented projection
        h_aug = z @ [W | W@a_src^T | W@a_dst^T]   (bf16, PSUM fp32)
    writing per-node rows [h(2304) | hi_s(3) | lo_s(3)] (attention source
    logits as a bf16 hi/lo pair, ~fp16+ precision). Destination logits stay
    resident in SBUF (per dst block, hi/lo).
  * Shard rows are AllGathered in NSPLIT row-chunks (pipelined with the
    projection) into a split-major layout; each core then holds all rows.
  * Edge phase per dst block: indirect-DMA gather of 128 src rows per chunk,
    e = leaky_relu(al_s[src] + al_d[dst]) (al_d expanded per edge by a
    mask-transpose matmul), p = exp(e) unnormalized (logits are O(10) so no
    max-subtraction needed; reference eps=1e-16 is negligible), segment sums
    via mask matmuls: s = M^T p and per-head (M*p_h)^T @ h_src accumulated
    into one packed PSUM tile, epilogue out = sum_h psum_h/(3 s_h) + bias
    (softmax normalization and head-mean commute with the edge sum), relu
    for layer 1.
  * The layer-2 projection of block b is emitted right after layer-1 edge
    block b (its input z stays in SBUF, transposed SBUF->SBUF by the DMA
    xbar), so the layer-2 AllGather chunks stream out while the layer-1
    edge phase is still running.

Self-contained: only numpy/ml_dtypes/concourse (environment packages).
"""

import os
from contextlib import ExitStack
from dataclasses import dataclass

import ml_dtypes
import numpy as np

import concourse.bass as bass
import concourse.mybir as mybir
import concourse.tile as tile
from concourse import bacc
from concourse.bass import IndirectOffsetOnAxis

F32 = mybir.dt.float32
BF16 = mybir.dt.bfloat16
I32 = mybir.dt.int32
AF = mybir.ActivationFunctionType
OP = mybir.AluOpType

P = 128
NEG_SLOPE = 0.2


@dataclass(frozen=True)
class Cfg:
    N: int = 50000           # nodes
    D: int = 768             # input feature dim
    H: int = 3               # heads
    C: int = 768             # per-head channels
    n_cores: int = 8
    NSPLIT: int = 7          # allgather row-chunks (in node tiles)

    @property
    def HC(self):
        return self.H * self.C

    @property
    def WCOLS(self):
        return self.HC + 2 * self.H  # W | Wa_src | Wa_dst

    @property
    def ROW(self):
        return self.HC + 2 * self.H  # h | hi_s | lo_s

    @property
    def SHARD(self):
        return self.N // self.n_cores

    @property
    def NT(self):
        return (self.SHARD + P - 1) // P

    @property
    def tile_splits(self):
        """Node-tile index groups per allgather chunk."""
        return [list(t) for t in
                np.array_split(np.arange(self.NT), min(self.NSPLIT, self.NT))]

    @property
    def splits(self):
        """Row ranges (start, size) of the shard per allgather chunk."""
        out = []
        for t in self.tile_splits:
            r0 = int(t[0]) * P
            r1 = min(int(t[-1] + 1) * P, self.SHARD)
            out.append((r0, r1 - r0))
        return out

    @property
    def col_tiles(self):
        ct, c0 = [], 0
        while c0 < self.WCOLS:
            ct.append((c0, min(512, self.WCOLS - c0)))
            c0 += 512
        return ct

    @property
    def head_segs(self):
        """Bank-aligned column segments of one head's [0, C) channels."""
        mt, c0 = [], 0
        while c0 < self.C:
            mt.append((c0, min(512, self.C - c0)))
            c0 += 512
        return mt


CFG = Cfg()


# ---------------------------------------------------------------- host prep


def _augment_weight(W, a_src, a_dst, cfg):
    Wa = np.zeros((cfg.D, cfg.WCOLS), np.float32)
    Wa[:, : cfg.HC] = W
    for h in range(cfg.H):
        Wa[:, cfg.HC + h] = W[:, h * cfg.C : (h + 1) * cfg.C] @ a_src[h]
        Wa[:, cfg.HC + cfg.H + h] = W[:, h * cfg.C : (h + 1) * cfg.C] @ a_dst[h]
    return Wa.astype(ml_dtypes.bfloat16)


def _hbf_pos(gid, cfg):
    """Map global node id -> row in the split-major allgathered table."""
    k = gid // cfg.SHARD
    r = gid % cfg.SHARD
    pos = np.zeros_like(gid)
    base = 0
    for (s0, sz) in cfg.splits:
        m = (r >= s0) & (r < s0 + sz)
        pos = np.where(m, base + k * sz + (r - s0), pos)
        base += cfg.n_cores * sz
    return pos


def _prep(x, edge_index, W1, a_src1, a_dst1, b1, W2, a_src2, a_dst2, b2, cfg):
    N, SHARD = cfg.N, cfg.SHARD
    NBLOCK = cfg.NT
    src = np.concatenate([np.asarray(edge_index[0]), np.arange(N)]).astype(np.int64)
    dst = np.concatenate([np.asarray(edge_index[1]), np.arange(N)]).astype(np.int64)
    order = np.argsort(dst, kind="stable")
    src_s, dst_s = src[order], dst[order]

    cores = []
    CPB = 1
    for k in range(cfg.n_cores):
        lo, hi = k * SHARD, (k + 1) * SHARD
        a = np.searchsorted(dst_s, lo)
        b = np.searchsorted(dst_s, hi)
        s_k = src_s[a:b].astype(np.int64)
        d_k = (dst_s[a:b] - lo).astype(np.int64)
        deg = np.bincount(d_k, minlength=SHARD)
        csum = np.concatenate([[0], np.cumsum(deg)])
        for bi in range(NBLOCK):
            n1 = min((bi + 1) * P, SHARD)
            ecnt = int(csum[n1] - csum[bi * P])
            CPB = max(CPB, (ecnt + P - 1) // P)
        cores.append((s_k, d_k, csum))
    NCHUNK = NBLOCK * CPB

    Wa1 = _augment_weight(np.asarray(W1, np.float32), np.asarray(a_src1, np.float32),
                          np.asarray(a_dst1, np.float32), cfg)
    Wa2 = _augment_weight(np.asarray(W2, np.float32), np.asarray(a_src2, np.float32),
                          np.asarray(a_dst2, np.float32), cfg)
    B1 = np.ascontiguousarray(
        np.broadcast_to(np.asarray(b1, np.float32), (P, cfg.C)))
    B2 = np.ascontiguousarray(
        np.broadcast_to(np.asarray(b2, np.float32), (P, cfg.C)))

    xf = np.asarray(x, np.float32)
    in_maps = []
    for k, (s_k, d_k, csum) in enumerate(cores):
        srcg = np.zeros((NCHUNK, P), np.int32)
        # per block: [M chunks | M^T chunks], one load per block
        MMc = np.zeros((NBLOCK, P, 2 * CPB * P), ml_dtypes.bfloat16)
        s_pos = _hbf_pos(s_k, cfg).astype(np.int32)
        for bi in range(NBLOCK):
            n1 = min((bi + 1) * P, SHARD)
            e0, e1_ = int(csum[bi * P]), int(csum[n1])
            ecnt = e1_ - e0
            es = s_pos[e0:e1_]
            ed = d_k[e0:e1_] - bi * P  # block-local dst slot
            eb = np.zeros(CPB * P, np.int32)
            eb[:ecnt] = es
            srcg[bi * CPB : (bi + 1) * CPB] = eb.reshape(CPB, P)
            Mb = np.zeros((CPB * P, P), ml_dtypes.bfloat16)
            Mb[np.arange(ecnt), ed] = 1.0
            Mb3 = Mb.reshape(CPB, P, P)
            MMc[bi, :, : CPB * P] = Mb3.transpose(1, 0, 2).reshape(P, CPB * P)
            MMc[bi, :, CPB * P :] = Mb3.transpose(2, 0, 1).reshape(P, CPB * P)
        xk = np.ascontiguousarray(
            xf[k * SHARD : (k + 1) * SHARD].T).astype(ml_dtypes.bfloat16)
        in_maps.append({
            "xT": xk,
            "W1a": Wa1, "W2a": Wa2, "B1": B1, "B2": B2,
            "SRC": np.ascontiguousarray(srcg.T),
            "MMC": np.ascontiguousarray(
                MMc.transpose(1, 0, 2).reshape(P, NBLOCK * 2 * CPB * P)),
        })
    return in_maps, CPB


# ---------------------------------------------------------------- device code


def _build(cfg, CPB):
    NBLOCK = cfg.NT
    NCHUNK = NBLOCK * CPB
    D, C, H, HC, WCOLS, ROW = cfg.D, cfg.C, cfg.H, cfg.HC, cfg.WCOLS, cfg.ROW
    SHARD, NT, N = cfg.SHARD, cfg.NT, cfg.N
    KT = D // P
    MW = 2 * CPB * P  # mask row-bytes per block (M | M^T)

    nc = bacc.Bacc("TRN2", target_bir_lowering=False, debug=False,
                   num_devices=cfg.n_cores)

    xT = nc.dram_tensor("xT", [D, SHARD], BF16, kind="ExternalInput")
    W1a = nc.dram_tensor("W1a", [D, WCOLS], BF16, kind="ExternalInput")
    W2a = nc.dram_tensor("W2a", [D, WCOLS], BF16, kind="ExternalInput")
    B1 = nc.dram_tensor("B1", [P, C], F32, kind="ExternalInput")
    B2 = nc.dram_tensor("B2", [P, C], F32, kind="ExternalInput")
    SRC = nc.dram_tensor("SRC", [P, NCHUNK], I32, kind="ExternalInput")
    MMC = nc.dram_tensor("MMC", [P, NBLOCK * MW], BF16, kind="ExternalInput")
    OUT = nc.dram_tensor("OUT", [SHARD, C], F32, kind="ExternalOutput")

    hbs = [nc.dram_tensor("hb1s", [SHARD, ROW], BF16),
           nc.dram_tensor("hb2s", [SHARD, ROW], BF16)]
    hbf = [nc.dram_tensor("hb1f", [N, ROW], BF16, addr_space="Shared"),
           nc.dram_tensor("hb2f", [N, ROW], BF16, addr_space="Shared")]

    groups = [list(range(cfg.n_cores))]
    split_rows = cfg.splits
    split_bases = np.cumsum([0] + [cfg.n_cores * sz for (_, sz) in split_rows])
    last_tile_of_split = {ts[-1]: j for j, ts in enumerate(cfg.tile_splits)}

    with tile.TileContext(nc) as tc, ExitStack() as ctx:
        res = ctx.enter_context(tc.tile_pool(name="res", bufs=1))
        b1_sb = res.tile([P, C], F32, name="b1_sb")
        nc.sync.dma_start(b1_sb[:], B1.ap())
        b2_sb = res.tile([P, C], F32, name="b2_sb")
        nc.sync.dma_start(b2_sb[:], B2.ap())
        src_sb = res.tile([P, NCHUNK], I32, name="src_sb")
        nc.sync.dma_start(src_sb[:], SRC.ap())
        ald1 = res.tile([P, NBLOCK * 2 * H], BF16, name="ald1")
        ald2 = res.tile([P, NBLOCK * 2 * H], BF16, name="ald2")
        aldr_ = [ald1, ald2]
        # pad partitions of the last block are never written but are read
        # (x0) by the expansion matmul — NaN garbage would poison it
        nc.gpsimd.memset(ald1[:], 0.0)
        nc.gpsimd.memset(ald2[:], 0.0)
        id_sb = res.tile([P, P], BF16, name="id_sb")
        from concourse.masks import make_identity
        make_identity(nc, id_sb[:])

        wp = ctx.enter_context(tc.tile_pool(name="wp", bufs=2))
        xp = ctx.enter_context(tc.tile_pool(name="xp", bufs=3))
        hp = ctx.enter_context(tc.tile_pool(name="hp", bufs=3))
        gp = ctx.enter_context(tc.tile_pool(name="gp", bufs=3))
        mp = ctx.enter_context(tc.tile_pool(name="mp", bufs=3))
        sp = ctx.enter_context(tc.tile_pool(name="sp", bufs=4))
        op_ = ctx.enter_context(tc.tile_pool(name="op", bufs=3))
        pp = ctx.enter_context(tc.tile_pool(name="pp", bufs=2, space="PSUM"))
        ep = ctx.enter_context(tc.tile_pool(name="ep", bufs=1, space="PSUM"))

        w_sbs = []
        for L in range(2):
            w_sb = wp.tile([P, KT * WCOLS], BF16, name=f"w_sb{L}", tag="W")
            nc.sync.dma_start(
                w_sb[:].rearrange("p (t c) -> p t c", t=KT),
                (W1a if L == 0 else W2a).ap().rearrange("(t p) c -> p t c", p=P))
            w_sbs.append(w_sb)

        def proj_tile(L, nt, z_sb=None):
            """Project node-tile nt of layer L: hbs[L] rows = z @ W_aug."""
            r0 = nt * P
            nw = min(P, SHARD - r0)
            lhsT = xp.tile([P, KT * P], BF16, name="lhsT", tag="lhsT")
            if L == 0:
                nc.sync.dma_start(
                    lhsT[:].rearrange("p (t n) -> p t n", t=KT)[:, :, :nw],
                    xT.ap()[:, r0 : r0 + nw].rearrange("(t p) n -> p t n", p=P))
            else:
                for kt in range(KT):
                    tp = ep.tile([P, P], BF16, name="tp", tag="ald")
                    nc.tensor.transpose(
                        tp[:], z_sb[:, kt * P : (kt + 1) * P], id_sb[:])
                    nc.vector.tensor_copy(
                        lhsT[:, kt * P : (kt + 1) * P], tp[:])
            hb_sb = hp.tile([P, ROW], BF16, name="hb_sb", tag="hb")
            aldr = aldr_[L]
            for (c0, cw) in cfg.col_tiles:
                ps = pp.tile([P, 512], F32, name="ps", tag="ps")
                for kt in range(KT):
                    nc.tensor.matmul(
                        ps[:nw, :cw],
                        lhsT=lhsT[:, kt * P : kt * P + nw],
                        rhs=w_sbs[L][:, kt * WCOLS + c0 : kt * WCOLS + c0 + cw],
                        start=(kt == 0), stop=(kt == KT - 1))
                if c0 + cw <= HC:
                    nc.scalar.copy(hb_sb[:nw, c0 : c0 + cw], ps[:nw, :cw])
                else:
                    la = HC - c0  # local col where al_s starts
                    nc.vector.tensor_copy(
                        hb_sb[:nw, c0 : HC + H], ps[:nw, : la + H])
                    nc.vector.tensor_tensor(
                        hb_sb[:nw, HC + H : HC + 2 * H],
                        ps[:nw, la : la + H],
                        hb_sb[:nw, HC : HC + H], op=OP.subtract)
                    ao = nt * 2 * H
                    nc.vector.tensor_copy(
                        aldr[:nw, ao : ao + H], ps[:nw, la + H : la + 2 * H])
                    nc.vector.tensor_tensor(
                        aldr[:nw, ao + H : ao + 2 * H],
                        ps[:nw, la + H : la + 2 * H],
                        aldr[:nw, ao : ao + H], op=OP.subtract)
            nc.sync.dma_start(hbs[L].ap()[r0 : r0 + nw, :], hb_sb[:nw, :])
            if nt in last_tile_of_split:
                j = last_tile_of_split[nt]
                s0, sz = split_rows[j]
                nc.gpsimd.collective_compute(
                    "AllGather", OP.bypass, replica_groups=groups,
                    ins=[hbs[L].ap()[s0 : s0 + sz, :].opt()],
                    outs=[hbf[L].ap()[int(split_bases[j]) :
                                      int(split_bases[j + 1]), :].opt()])

        def edge_block(L, b):
            """Edge aggregation for dst block b of layer L. Returns the
            bf16 activation tile (layer 0) after relu, or None (layer 1,
            writes OUT)."""
            nw = min(P, SHARD - b * P)
            s_ps = ep.tile([P, H], F32, name="s_ps", tag="s")
            Mc = mp.tile([P, MW], BF16, name="Mc", tag="M")
            nc.sync.dma_start(Mc[:], MMC.ap()[:, b * MW : (b + 1) * MW])
            aldr = aldr_[L]
            Gs, pfs = [], []
            for cc in range(CPB):
                c = b * CPB + cc
                G = gp.tile([P, ROW], BF16, name="G", tag="G", bufs=CPB + 2)
                nc.gpsimd.indirect_dma_start(
                    out=G[:], out_offset=None, in_=hbf[L].ap(),
                    in_offset=IndirectOffsetOnAxis(
                        ap=src_sb[:, c : c + 1], axis=0))
                Mt = Mc[:, cc * P : (cc + 1) * P]
                aldt = ep.tile([P, 2 * H], F32, name="aldt", tag="ald")
                nc.tensor.matmul(
                    aldt[:], lhsT=Mc[:, CPB * P + cc * P : CPB * P + (cc + 1) * P],
                    rhs=aldr[:, b * 2 * H : (b + 1) * 2 * H],
                    start=True, stop=True)
                e1 = sp.tile([P, H], F32, name="e1", tag="e1")
                nc.vector.tensor_tensor(
                    e1[:], G[:, HC : HC + H], G[:, HC + H : HC + 2 * H],
                    op=OP.add)
                nc.vector.tensor_tensor(e1[:], e1[:], aldt[:, 0:H], op=OP.add)
                nc.vector.tensor_tensor(
                    e1[:], e1[:], aldt[:, H : 2 * H], op=OP.add)
                nc.vector.scalar_tensor_tensor(
                    e1[:], e1[:], NEG_SLOPE, e1[:], op0=OP.mult, op1=OP.max)
                pf = sp.tile([P, H], F32, name="pf", tag="pf", bufs=CPB + 2)
                nc.scalar.activation(pf[:], e1[:], AF.Exp)
                pb = sp.tile([P, H], BF16, name="pb", tag="pb")
                nc.vector.tensor_copy(pb[:], pf[:])
                nc.tensor.matmul(
                    s_ps[:], lhsT=Mt, rhs=pb[:],
                    start=(cc == 0), stop=(cc == CPB - 1))
                Gs.append(G)
                pfs.append(pf)
            s_sb = sp.tile([P, H], F32, name="s_sb", tag="s_sb")
            nc.vector.tensor_copy(s_sb[:], s_ps[:])
            recip = sp.tile([P, H], F32, name="recip", tag="recip")
            nc.vector.tensor_scalar_mul(recip[:], s_sb[:], float(H))
            nc.vector.reciprocal(recip[:], recip[:])
            bias_sb = b1_sb if L == 0 else b2_sb
            o = op_.tile([P, C], F32, name="o", tag="o")
            for h in range(H):
                hacc = ep.tile([P, C], F32, name="hacc", tag="hacc", bufs=2)
                for cc in range(CPB):
                    Sh = mp.tile([P, P], BF16, name="Sh", tag="Sh", bufs=6)
                    nc.vector.tensor_scalar_mul(
                        Sh[:], Mc[:, cc * P : (cc + 1) * P],
                        pfs[cc][:, h : h + 1])
                    for (a, w_) in cfg.head_segs:
                        nc.tensor.matmul(
                            hacc[:, a : a + w_], lhsT=Sh[:],
                            rhs=Gs[cc][:, h * C + a : h * C + a + w_],
                            start=(cc == 0), stop=(cc == CPB - 1))
                nc.vector.scalar_tensor_tensor(
                    o[:], hacc[:], recip[:, h : h + 1],
                    bias_sb[:] if h == 0 else o[:],
                    op0=OP.mult, op1=OP.add)
            if L == 0:
                z = op_.tile([P, C], BF16, name="z", tag="z")
                nc.scalar.activation(z[:], o[:], AF.Relu)
                return z
            nc.sync.dma_start(OUT.ap()[b * P : b * P + nw, :], o[:nw, :])
            return None

        # layer-1 projection (allgather chunks emitted inline)
        for nt in range(NT):
            proj_tile(0, nt)
        # layer-1 edge phase interleaved with layer-2 projection
        for b in range(NBLOCK):
            z = edge_block(0, b)
            proj_tile(1, b, z_sb=z)
        # layer-2 edge phase
        for b in range(NBLOCK):
            edge_block(1, b)

    nc.compile()
    return nc


# ---------------------------------------------------------------- entry point

_NC_CACHE = {}


def _get_nc(cfg, CPB):
    key = (cfg, CPB)
    if key not in _NC_CACHE:
        _NC_CACHE[key] = _build(cfg, CPB)
    return _NC_CACHE[key]


LAST_RUN = {}


def kernel(x, edge_index, W1, a_src1, a_dst1, b1, W2, a_src2, a_dst2, b2,
           cfg=CFG):
    from concourse.bass_utils import run_bass_kernel_spmd

    in_maps, CPB = _prep(x, edge_index, W1, a_src1, a_dst1, b1,
                         W2, a_src2, a_dst2, b2, cfg)
    nc = _get_nc(cfg, CPB)
    trace = os.environ.get("GAT_TRACE", "0") == "1"
    tmpdir = os.environ.get("GAT_TMPDIR") or None
    res = run_bass_kernel_spmd(nc, in_maps, list(range(cfg.n_cores)),
                               trace=trace, tmpdir=tmpdir)
    LAST_RUN["exec_time_ns"] = res.exec_time_ns
    LAST_RUN["profile_json"] = res.profile_json
    out = np.concatenate(
        [res.results[k]["OUT"] for k in range(cfg.n_cores)], 0)
    return np.ascontiguousarray(out.astype(np.float32))
